# revision 4
# baseline (speedup 1.0000x reference)
"""Trainium2 Bass kernel for nn_GRNNTransformGated (recursive tree GRNN over
1024 independent 10-level binary jets).

Strategy (v2):
  - Data-parallel over jets: 8 cores x 128 trees each.
  - BIT-REVERSED per-tree node layout: storing level j in bit-reversed local
    order makes the two children of parent position q land at positions q and
    q + 2^j of the level below -- every child "gather" becomes two contiguous
    stride-1 slices, so all DVE ops run in packed bf16 2x mode.
  - Softmax shift-invariance: z gates computed as differences d_m = z_m - z_3,
    so the z matmul is 4Hx3H (12 matmuls) instead of 4Hx4H (16), only 3 exps,
    e3 == 1 (numerator gets +u, denominator gets +1).
  - Feature-major layout [128 channels (partitions), nodes (free)]; weight
    blocks stationary, 25 matmuls per 512-node tile.
  - 8 separate PSUM banks (pu, pr0-2, ph, pz0-2) so consecutive tiles overlap
    with only early-read WAR dependencies -- keeps the PE dense and the HAM
    clock-gate warm (K=8/8).
  - conv_chain collapses: for w>0, b>=0, f(f(f(x))) = w^2*relu(w*x+b) + (w*b+b).
  - sigmoid via tanh; the 0.5 is folded into W_h on the host.
  - Per-tree interleaved temporaries (t01, e12, p12) let pairs of elementwise
    ops fuse into single FD=1024 stride-1 instructions.
"""

import sys

for _p in ("/opt/trn_rl_repo", "/root/.axon_site/_ro/trn_rl_repo"):
    if _p not in sys.path:
        sys.path.insert(0, _p)

import numpy as np

B = 1024
L = 10
H = 128
FEAT = 7
NCORES = 8
TPC = B // NCORES          # trees per core = 128
TCH = 16                   # trees per chunk
NCHUNK = TPC // TCH        # 8 chunks
NPC = TPC * (2 ** L - 1)   # nodes per core = 130944
LOFF = [TPC * (2 ** j - 1) for j in range(L + 1)]  # level offsets in per-core ct
LEVEL_SIZES = [B * 2 ** j for j in range(L)]
OFF = np.concatenate([[0], np.cumsum(LEVEL_SIZES)]).astype(int)
INNER = LEVEL_SIZES[:-1]
COFF = np.concatenate([[0], np.cumsum(INNER)]).astype(int)

MMT = 512  # matmul node-tile size

_CACHE = {}


def _children_canonical(children):
    for j in range(L - 1):
        n = INNER[j]
        blk = children[COFF[j]:COFF[j + 1]]
        base = 2 * np.arange(n, dtype=np.int64)
        if not (np.array_equal(blk[:, 0], base) and np.array_equal(blk[:, 1], base + 1)):
            return False
    return True


def _numpy_fallback(contents, children, W_u, b_u, W_h, b_h, W_z, b_z, W_r, b_r,
                    conv_w, conv_b):
    w, b = float(conv_w[0]), float(conv_b[0])

    def conv_chain(x):
        for _ in range(3):
            x = np.maximum(w * x + b, 0.0)
        return x

    def sigmoid(x):
        return 1.0 / (1.0 + np.exp(-x))

    emb = None
    for j in reversed(range(L)):
        c = contents[OFF[j]:OFF[j + 1]]
        u = conv_chain(c @ W_u + b_u)
        if j == L - 1:
            emb = u
            continue
        ch = children[COFF[j]:COFF[j + 1]]
        h_L = emb[ch[:, 0]]
        h_R = emb[ch[:, 1]]
        hhu = np.concatenate([h_L, h_R, u], axis=1)
        r = sigmoid(hhu @ W_r + b_r)
        h_H = conv_chain((r * hhu) @ W_h + b_h)
        z = np.concatenate([h_H, hhu], axis=1) @ W_z + b_z
        zs = np.stack([z[:, :H], z[:, H:2 * H], z[:, 2 * H:3 * H], z[:, 3 * H:]], axis=-1)
        zs = zs - zs.max(axis=-1, keepdims=True)
        e = np.exp(zs)
        g = e / e.sum(axis=-1, keepdims=True)
        emb = g[..., 0] * h_H + g[..., 1] * h_L + g[..., 2] * h_R + g[..., 3] * u
    return emb.reshape(B, -1).astype(np.float32)


def _bitrev_perm(j):
    """perm[q] = bit-reverse of q over j bits."""
    if j == 0:
        return np.zeros(1, dtype=np.int64)
    return (
        np.arange(2 ** j, dtype=np.int64)
        .reshape((2,) * j)
        .transpose(tuple(reversed(range(j))))
        .ravel()
    )


def _build(cw, cb, collapsible, do_affine, A, C):
    from contextlib import ExitStack

    from concourse import bacc, bass, mybir, tile

    f32 = mybir.dt.float32
    bf16 = mybir.dt.bfloat16
    AF = mybir.ActivationFunctionType
    OP = mybir.AluOpType

    nc = bacc.Bacc()

    ct_d = nc.declare_dram_parameter("ct", [FEAT, NPC], bf16, isOutput=False)
    wu_d = nc.declare_dram_parameter("wu", [FEAT, H], bf16, isOutput=False)
    wr_d = nc.declare_dram_parameter("wr", [H, 3, 3, H], bf16, isOutput=False)
    wh_d = nc.declare_dram_parameter("wh", [H, 3, H], bf16, isOutput=False)
    wz_d = nc.declare_dram_parameter("wz", [H, 4, 3, H], bf16, isOutput=False)
    bv_d = nc.declare_dram_parameter("bvec", [H, 8], f32, isOutput=False)
    id_d = nc.declare_dram_parameter("ident", [H, H], f32, isOutput=False)
    out_d = nc.declare_dram_parameter("out", [TPC, H], f32, isOutput=True)

    with ExitStack() as ctx:
        tc = ctx.enter_context(tile.TileContext(nc))
        wpool = ctx.enter_context(tc.tile_pool(name="wts", bufs=1))
        epool = ctx.enter_context(tc.tile_pool(name="emb", bufs=1))
        ctpool = ctx.enter_context(tc.tile_pool(name="ct", bufs=3))
        spool = ctx.enter_context(tc.tile_pool(name="tmp", bufs=3))
        pp = ctx.enter_context(tc.tile_pool(name="ps", bufs=1, space="PSUM"))

        wu = wpool.tile([FEAT, H], bf16, tag="wu")
        wr = wpool.tile([H, 3, 3, H], bf16, tag="wr")
        wh = wpool.tile([H, 3, H], bf16, tag="wh")
        wz = wpool.tile([H, 4, 3, H], bf16, tag="wz")
        bv = wpool.tile([H, 8], f32, tag="bv")
        idt = wpool.tile([H, H], f32, tag="idt")
        nc.sync.dma_start(wu[:], wu_d[:])
        nc.sync.dma_start(wr[:], wr_d[:])
        nc.sync.dma_start(wh[:], wh_d[:])
        nc.sync.dma_start(wz[:], wz_d[:])
        nc.sync.dma_start(bv[:], bv_d[:])
        nc.sync.dma_start(idt[:], id_d[:])

        # emb level buffers (phase A holds one chunk; emb5 accumulates all chunks)
        e9 = epool.tile([H, TCH * 512], bf16, tag="e9")     # 8192
        e8 = epool.tile([H, TCH * 256], bf16, tag="e8")     # 4096
        e7 = epool.tile([H, TCH * 128], bf16, tag="e7")     # 2048
        e6 = epool.tile([H, TCH * 64], bf16, tag="e6")      # 1024
        emb5 = epool.tile([H, TPC * 32], bf16, tag="emb5")  # 4096 (all trees)

        def conv_tail(dst):
            if collapsible:
                if do_affine:
                    nc.vector.tensor_scalar(dst, dst, A, C, OP.mult, OP.add)
            else:
                nc.scalar.activation(dst, dst, AF.Relu, bias=cb, scale=cw)
                nc.scalar.activation(dst, dst, AF.Relu, bias=cb, scale=cw)

        def inner_tile(cbuf, w, s, n, ct_ap, out_ap):
            """One tile of n parents at level offset s; parents span whole
            trees (n = ntt*w). cbuf is the child-level buffer (per-tree width
            2w, bit-reversed halves)."""
            ntt = n // w
            t0 = s // w
            cb4 = cbuf.rearrange("p (t two w) -> p t two w", two=2, w=w)
            hL = cb4[:, t0:t0 + ntt, 0, :]                 # [H, ntt, w]
            hR = cb4[:, t0:t0 + ntt, 1, :]
            cb_both = cbuf[:, 2 * s:2 * s + 2 * n]          # [H, 2n] contiguous

            # ---- u ----
            pu = pp.tile([H, MMT], f32, name="pu", tag="pu")
            nc.tensor.matmul(pu[:, :n], wu[:], ct_ap, start=True, stop=True)
            up = spool.tile([H, MMT], bf16, name="up", tag="up")
            u = up[:, :n]
            nc.scalar.activation(u, pu[:, :n], AF.Relu, bias=bv[:, 0:1], scale=cw)
            conv_tail(u)
            # ---- r gates (as tanh), t0/t1 interleaved per tree ----
            prs = [pp.tile([H, MMT], f32, name=f"pr{m}", tag=f"pr{m}") for m in range(3)]
            rhs_k = [hL, hR, u]
            for m in range(3):
                for k in range(3):
                    nc.tensor.matmul(prs[m][:, :n], wr[:, k, m, :], rhs_k[k],
                                     start=(k == 0), stop=(k == 2))
            t01 = spool.tile([H, 2 * MMT], bf16, name="t01", tag="t01")
            t01v = t01.rearrange("p (t two w) -> p t two w", two=2, w=w)
            t2 = spool.tile([H, MMT], bf16, name="t2", tag="t2")
            for m in range(2):
                nc.scalar.activation(t01v[:, :ntt, m, :], prs[m][:, :n], AF.Tanh,
                                     bias=bv[:, 1 + m:2 + m], scale=0.5)
            nc.scalar.activation(t2[:, :n], prs[2][:, :n], AF.Tanh,
                                 bias=bv[:, 3:4], scale=0.5)
            # ---- rh = (t+1) * hhu   (x0.5 folded into W_h) ----
            rh01 = spool.tile([H, 2 * MMT], bf16, name="rh01", tag="rh01")
            nc.vector.scalar_tensor_tensor(rh01[:, :2 * n], t01[:, :2 * n], 1.0,
                                           cb_both, OP.add, OP.mult)
            rh2 = spool.tile([H, MMT], bf16, name="rh2", tag="rh2")
            nc.vector.scalar_tensor_tensor(rh2[:, :n], t2[:, :n], 1.0,
                                           u, OP.add, OP.mult)
            rh01v = rh01.rearrange("p (t two w) -> p t two w", two=2, w=w)
            ph = pp.tile([H, MMT], f32, name="ph", tag="ph")
            nc.tensor.matmul(ph[:, :n], wh[:, 0, :], rh01v[:, :ntt, 0, :],
                             start=True, stop=False)
            nc.tensor.matmul(ph[:, :n], wh[:, 1, :], rh01v[:, :ntt, 1, :],
                             start=False, stop=False)
            nc.tensor.matmul(ph[:, :n], wh[:, 2, :], rh2[:, :n],
                             start=False, stop=True)
            hp = spool.tile([H, MMT], bf16, name="hp", tag="hp")
            hH = hp[:, :n]
            nc.scalar.activation(hH, ph[:, :n], AF.Relu, bias=bv[:, 4:5], scale=cw)
            conv_tail(hH)
            # ---- z diffs d_m = z_m - z_3, m=0..2 ----
            zk = [hH, hL, hR, u]
            pzs = [pp.tile([H, MMT], f32, name=f"pz{m}", tag=f"pz{m}") for m in range(3)]
            for m in range(3):
                for k in range(4):
                    nc.tensor.matmul(pzs[m][:, :n], wz[:, k, m, :], zk[k],
                                     start=(k == 0), stop=(k == 3))
            e0 = spool.tile([H, MMT], bf16, name="e0", tag="e0")
            e12 = spool.tile([H, 2 * MMT], bf16, name="e12", tag="e12")
            e12v = e12.rearrange("p (t two w) -> p t two w", two=2, w=w)
            nc.scalar.activation(e0[:, :n], pzs[0][:, :n], AF.Exp, bias=bv[:, 5:6])
            nc.scalar.activation(e12v[:, :ntt, 0, :], pzs[1][:, :n], AF.Exp,
                                 bias=bv[:, 6:7])
            nc.scalar.activation(e12v[:, :ntt, 1, :], pzs[2][:, :n], AF.Exp,
                                 bias=bv[:, 7:8])
            # ---- denominator s = 1 + e0 + e1 + e2 (gpsimd adds), rcp (vector) ----
            s1 = spool.tile([H, MMT], bf16, name="s1", tag="s1")
            nc.gpsimd.tensor_tensor(s1[:, :n], e0[:, :n],
                                    e12v[:, :ntt, 0, :], OP.add)
            s2 = spool.tile([H, MMT], bf16, name="s2", tag="s2")
            nc.gpsimd.tensor_tensor(s2[:, :n], s1[:, :n],
                                    e12v[:, :ntt, 1, :], OP.add)
            sf = spool.tile([H, MMT], f32, name="sf", tag="sf")
            nc.vector.tensor_scalar(sf[:, :n], s2[:, :n], 1.0, 1.0,
                                    OP.mult, OP.add)
            rcp = spool.tile([H, MMT], f32, name="rcp", tag="rcp")
            nc.vector.reciprocal_approx_fast(rcp[:, :n], sf[:, :n])
            # ---- numerator = e0*hH + e1*hL + e2*hR + u ----
            p12 = spool.tile([H, 2 * MMT], bf16, name="p12", tag="p12")
            nc.vector.tensor_tensor(p12[:, :2 * n], e12[:, :2 * n], cb_both, OP.mult)
            p0 = spool.tile([H, MMT], bf16, name="p0", tag="p0")
            nc.vector.tensor_tensor(p0[:, :n], e0[:, :n], hH, OP.mult)
            p12v = p12.rearrange("p (t two w) -> p t two w", two=2, w=w)
            bb = spool.tile([H, MMT], bf16, name="bb", tag="bb")
            bbv = bb.rearrange("p (t w) -> p t w", w=w)
            nc.gpsimd.tensor_tensor(bbv[:, :ntt, :], p12v[:, :ntt, 0, :],
                                    p12v[:, :ntt, 1, :], OP.add)
            aa = spool.tile([H, MMT], bf16, name="aa", tag="aa")
            nc.vector.tensor_tensor(aa[:, :n], p0[:, :n], u, OP.add)
            num = spool.tile([H, MMT], bf16, name="num", tag="num")
            nc.vector.tensor_tensor(num[:, :n], aa[:, :n], bb[:, :n], OP.add)
            nc.vector.tensor_tensor(out_ap, num[:, :n], rcp[:, :n], OP.mult)

        def run_level(nj, w, ct_base, cbuf, obuf):
            """One level with nj parents of per-tree width w."""
            done = 0
            while done < nj:
                piece = min(2048, nj - done)
                ctt = ctpool.tile([FEAT, 2048], bf16, name="ctt", tag="ctt")
                nc.sync.dma_start(ctt[:, :piece],
                                  ct_d[:, ct_base + done:ct_base + done + piece])
                for s in range(0, piece, MMT):
                    n = min(MMT, piece - s)
                    base = done + s
                    inner_tile(cbuf, w, base, n, ctt[:, s:s + n],
                               obuf[:, base:base + n])
                done += piece

        # ================= phase A: per-chunk levels 9..5 =================
        for c in range(NCHUNK):
            # leaf level 9
            nleaf = TCH * 512  # 8192
            base9 = LOFF[9] + c * nleaf
            for hpiece in range(0, nleaf, 2048):
                ctt = ctpool.tile([FEAT, 2048], bf16, name="ctt", tag="ctt")
                nc.sync.dma_start(ctt[:], ct_d[:, base9 + hpiece:base9 + hpiece + 2048])
                for s in range(0, 2048, MMT):
                    pu = pp.tile([H, MMT], f32, name="pu", tag="pu")
                    nc.tensor.matmul(pu[:], wu[:], ctt[:, s:s + MMT],
                                     start=True, stop=True)
                    dst = e9[:, hpiece + s:hpiece + s + MMT]
                    nc.scalar.activation(dst, pu[:], AF.Relu,
                                         bias=bv[:, 0:1], scale=cw)
                    if not collapsible:
                        nc.scalar.activation(dst, dst, AF.Relu, bias=cb, scale=cw)
                        nc.scalar.activation(dst, dst, AF.Relu, bias=cb, scale=cw)
                if collapsible and do_affine:
                    big = e9[:, hpiece:hpiece + 2048]
                    nc.vector.tensor_scalar(big, big, A, C, OP.mult, OP.add)
            # inner levels 8..5
            for j, (cbuf, obuf) in zip(
                    range(8, 4, -1),
                    [(e9, e8), (e8, e7), (e7, e6), (e6, None)]):
                nj = TCH * (2 ** j)
                if j == 5:
                    ob = emb5[:, c * 512:(c + 1) * 512]
                else:
                    ob = obuf[:, :nj]
                run_level(nj, 2 ** j, LOFF[j] + c * nj, cbuf[:], ob)

        # ================= phase B: levels 4..0, all trees =================
        # reuse dead phase-A buffers for the tail levels
        e4 = e8[:, :2048]
        e3 = e7[:, :1024]
        e2 = e6[:, :512]
        e1 = e8[:, 2048:2048 + 256]
        e0f = epool.tile([H, TPC], f32, tag="e0f")
        e0t = e0f[:, :TPC]
        chain = [(emb5[:], e4), (e4, e3), (e3, e2), (e2, e1), (e1, e0t)]
        for j, (cbap, ob) in zip(range(4, -1, -1), chain):
            nj = TPC * (2 ** j)
            run_level(nj, 2 ** j, LOFF[j], cbap, ob)

        # ================= output transpose + store =================
        pt = pp.tile([H, H], f32, name="pz0", tag="pz0")
        nc.tensor.matmul(pt[:], e0t, idt[:], is_transpose=True, start=True, stop=True)
        osb = spool.tile([H, H], f32, name="osb", tag="osb")
        nc.vector.tensor_copy(osb[:], pt[:])
        nc.sync.dma_start(out_d[:], osb[:])

    nc.compile()
    if not nc.is_finalized():
        nc.finalize()
    return nc


def _prepare(inputs):
    contents = np.ascontiguousarray(np.asarray(inputs["contents"], np.float32))
    W_u = np.asarray(inputs["W_u"], np.float32)
    b_u = np.asarray(inputs["b_u"], np.float32)
    W_h = np.asarray(inputs["W_h"], np.float32)
    b_h = np.asarray(inputs["b_h"], np.float32)
    W_z = np.asarray(inputs["W_z"], np.float32)
    b_z = np.asarray(inputs["b_z"], np.float32)
    W_r = np.asarray(inputs["W_r"], np.float32)
    b_r = np.asarray(inputs["b_r"], np.float32)
    cw = float(np.asarray(inputs["conv_w"]).reshape(-1)[0])
    cb = float(np.asarray(inputs["conv_b"]).reshape(-1)[0])

    # per-core feature-major contents, level-major columns, bit-reversed
    # per-tree node order within each level
    cts = np.empty((NCORES, FEAT, NPC), np.float32)
    col = 0
    for j in range(L):
        n = TPC * 2 ** j
        blk = contents[OFF[j]:OFF[j + 1]].reshape(NCORES, TPC, 2 ** j, FEAT)
        if j > 0:
            blk = blk[:, :, _bitrev_perm(j), :]
        blk = blk.reshape(NCORES, n, FEAT)
        cts[:, :, col:col + n] = blk.transpose(0, 2, 1)
        col += n

    wr_np = np.ascontiguousarray(W_r.reshape(3, H, 3, H).transpose(1, 0, 2, 3))
    wh_np = np.ascontiguousarray((0.5 * W_h).reshape(3, H, H).transpose(1, 0, 2))
    # z-diff weights: Wd[k, :, m, :] = W_z[k,:,m,:] - W_z[k,:,3,:] for m=0..2
    wz4 = W_z.reshape(4, H, 4, H)
    wzd = np.ascontiguousarray(
        (wz4[:, :, 0:3, :] - wz4[:, :, 3:4, :]).transpose(1, 0, 2, 3))

    bvec = np.zeros((H, 8), np.float32)
    bvec[:, 0] = cw * b_u + cb
    bvec[:, 1:4] = 0.5 * b_r.reshape(3, H).T
    bvec[:, 4] = cw * b_h + cb
    bz4 = b_z.reshape(4, H)
    bvec[:, 5:8] = (bz4[0:3] - bz4[3:4]).T

    import ml_dtypes

    bf = ml_dtypes.bfloat16
    common = {
        "wu": np.ascontiguousarray(W_u).astype(bf),
        "wr": wr_np.astype(bf), "wh": wh_np.astype(bf), "wz": wzd.astype(bf),
        "bvec": bvec,
        "ident": np.eye(H, dtype=np.float32),
    }
    in_maps = [dict(common, ct=np.ascontiguousarray(cts[c]).astype(bf))
               for c in range(NCORES)]
    return in_maps, cw, cb


def kernel(**inputs):
    children = np.asarray(inputs["children"])
    cw = float(np.asarray(inputs["conv_w"]).reshape(-1)[0])
    cb = float(np.asarray(inputs["conv_b"]).reshape(-1)[0])
    collapsible = (cw >= 0.0) and (cb >= 0.0)
    if not _children_canonical(children):
        args = {k: np.asarray(v) for k, v in inputs.items()}
        return _numpy_fallback(**args)

    from concourse.bass_utils import run_bass_kernel_spmd

    A = cw * cw
    C = cw * cb + cb
    do_affine = not (A == 1.0 and C == 0.0)

    key = (cw, cb, collapsible, do_affine)
    if key not in _CACHE:
        _CACHE[key] = _build(cw, cb, collapsible, do_affine, A, C)
    nc = _CACHE[key]

    in_maps, _, _ = _prepare(inputs)
    res = run_bass_kernel_spmd(nc, in_maps, list(range(NCORES)))
    outs = [res.results[c]["out"] for c in range(NCORES)]
    return np.ascontiguousarray(np.concatenate(outs, axis=0).astype(np.float32))


if __name__ == "__main__":
    rng = np.random.default_rng(0)
    print("kernel module loaded")
